# revision 51
# baseline (speedup 1.0000x reference)
"""Tensor-parallel multi-head attention for Trainium2 (8 NeuronCores).

Problem: nn_MultiHeadAttention (B=2, N=2048, C=1024, H=16, D=64), fp32 in/out.

Sharding: core = batch * 4 + head_group; each core handles 1 batch and 4
heads (tensor-parallel over heads, data-parallel over batch). Each core
computes its heads' QKV projections, attention, and a *partial* output
projection (its 256 rows of w_proj); the host sums the 4 partials per
batch and adds b_proj.

Kernel notes (bf16 matmuls, fp32 PSUM accumulation + softmax chain):
  - The Scalar engine's exp stream is the hard floor (1 elem/cycle/lane,
    dtype-independent): 128 ACTIVATEs of [128,1024] ~= 147us. Everything
    else is scheduled around keeping that stream dense.
  - x is transposed on the host to xT [C, N] (bf16); xT is DMA'd in
    token-blocks (block 0 chunk-by-chunk) so the first k/q projection
    groups stream in as data lands and the first exp fires early.
  - Scores are computed transposed: sT[m, n] = k[m].q[n], keys m on
    partitions -- P@V then needs no transposes. Two heads run
    concurrently on the PE via row-tiling (K=64 each).
  - Softmax: no max-subtraction (logits ~ N(0,1)); the denominator is
    obtained by appending a ones-column to V (row 64 of the [65,512]
    attention-output accumulators); probabilities are normalized after
    the P@V matmuls via a DRAM-bounce partition-broadcast + fast
    reciprocal; v-bias is added then (softmax rows sum to 1); k-bias is
    softmax-invariant and dropped; q-bias applied at QKV eviction;
    proj-bias added on the host.
  - Background work (V projection, k/q groups for later passes) drips
    into the attention slots in small doses; V uses a private PSUM bank
    (shared with the projection later -- disjoint phases) so its ring
    never couples with the k/q group ring.
  - The output projection mostly runs in the tail, where the scores
    pool is dead: wide [128,1024] PSUM tiles, evictions alternating
    between the two then-idle copy engines.
"""

import numpy as np
from contextlib import ExitStack

P = 128
C = 1024
D = 64
N_CORES = 8

_BUILT = {}
TRACE = False   # set True (e.g. from test.py) to capture an NTFF profile
LAST_RESULTS = None  # BassKernelResults of the most recent kernel() call


def _build(n_tok, debug=False):
    import concourse.bass as bass
    import concourse.mybir as mybir
    import concourse.tile as tile
    from concourse import bacc
    from concourse.bass import ts

    fp32 = mybir.dt.float32
    bf16 = mybir.dt.bfloat16
    Exp = mybir.ActivationFunctionType.Exp
    Copy = mybir.ActivationFunctionType.Copy
    mult = mybir.AluOpType.mult

    NC5 = n_tok // 512  # 512-wide query chunks
    MC = n_tok // 128   # 128-wide key chunks
    CC = C // P         # contraction chunks for projections

    nc = bacc.Bacc("TRN2", target_bir_lowering=False, debug=debug)

    xt_d = nc.dram_tensor("xt", [C, n_tok], bf16, kind="ExternalInput").ap()
    wqk_d = nc.dram_tensor("w_qk", [C, 512], bf16, kind="ExternalInput").ap()
    wv_d = nc.dram_tensor("w_v", [C, 256], bf16, kind="ExternalInput").ap()
    wp_d = nc.dram_tensor("w_p", [256, C], bf16, kind="ExternalInput").ap()
    bq_d = nc.dram_tensor("b_q", [256], fp32, kind="ExternalInput").ap()
    bv_d = nc.dram_tensor("b_v2", [64, 4], fp32, kind="ExternalInput").ap()
    out_d = nc.dram_tensor("out", [n_tok, C], fp32, kind="ExternalOutput").ap()

    with tile.TileContext(nc) as tc, ExitStack() as ctx:
        persist = ctx.enter_context(tc.tile_pool(name="persist", bufs=1))
        p_pool = ctx.enter_context(tc.tile_pool(name="p_pool", bufs=4))
        ev_pool = ctx.enter_context(tc.tile_pool(name="ev_pool", bufs=2))
        late = ctx.enter_context(tc.tile_pool(name="late", bufs=3))
        s_pool = ctx.enter_context(tc.tile_pool(name="s", bufs=2, space="PSUM"))
        o2_pool = ctx.enter_context(tc.tile_pool(name="o2", bufs=2, space="PSUM"))
        sm_pool = ctx.enter_context(tc.tile_pool(name="sm", bufs=1, space="PSUM"))
        pp_pool = ctx.enter_context(tc.tile_pool(name="pp", bufs=1, space="PSUM"))
        dram_pool = ctx.enter_context(tc.tile_pool(name="dram", bufs=4, space="DRAM"))

        xt = persist.tile([P, CC, n_tok], bf16)
        wqk = persist.tile([P, CC, 512], bf16)
        wv = persist.tile([P, CC, 256], bf16)
        bq = persist.tile([P, 2], fp32)
        wp = persist.tile([P, 2, C], bf16)
        bv = persist.tile([64, 4], fp32)
        qk = persist.tile([P, 4, n_tok], bf16)   # jc: 0,1 = qT pairs, 2,3 = kT pairs
        vsb = persist.tile([P, MC, 4, 65], bf16)  # token-major V + ones column
        o2n = persist.tile([P, 2, n_tok], bf16)  # normalized attn out, feature-major

        # DMA dispatch costs ~600ns/instruction, so keep the count low:
        # pair-0 weight columns first, then xt token-block 0 chunk-by-chunk
        # (per-chunk sems let the first k/q groups drip as data arrives),
        # then the rest as whole-block transfers; pair-1 weights and w_p
        # (not needed until attn(1)/proj) go last.
        # All transfers use whole DRAM rows per contraction chunk (one
        # contiguous multi-KB run per partition -> ~600ns dispatch each).
        # xt block 0 goes first in per-chunk pieces so the first k/q
        # groups stream in as chunks land; the row-remainders follow.
        xt_src = xt_d.rearrange("(co p) n -> p co n", p=P)
        wqk_src = wqk_d.rearrange("(co p) j -> p co j", p=P)
        nc.gpsimd.dma_start(bq[:], bq_d.rearrange("(c p) -> p c", p=P))
        nc.gpsimd.dma_start(bv[:], bv_d)
        for cc in range(CC):   # wqk whole rows (all of q/k, both pairs)
            eng = nc.sync if (cc % 2 == 0) else nc.scalar
            eng.dma_start(wqk[:, cc, :], wqk_src[:, cc, :])
        for cc in range(CC):   # xt block 0, per-chunk
            eng = nc.sync if (cc % 2 == 0) else nc.scalar
            eng.dma_start(xt[:, cc, 0:512], xt_src[:, cc, 0:512])
        nc.gpsimd.dma_start(wv[:], wv_d.rearrange("(co p) j -> p co j", p=P))
        for cc in range(CC):   # xt row remainders (tokens 512:end)
            eng = nc.sync if (cc % 2 == 0) else nc.scalar
            eng.dma_start(xt[:, cc, 512:n_tok], xt_src[:, cc, 512:n_tok])
        nc.gpsimd.dma_start(wp[:], wp_d.rearrange("(pc p) e -> p pc e", p=P))
        ones = persist.tile([P, 1], fp32)
        nc.vector.memset(ones[:], 1.0)
        nc.vector.tensor_copy(
            out=vsb[:, :, :, 64:65],
            in_=ones[:, None, :, None].to_broadcast((P, MC, 4, 1)),
        )

        def qk_part(jc, wcol, n5, state, lo, hi):
            # chunks [lo, hi) of one 512-wide projection group; evict at hi==CC
            if "ps" not in state:
                state["ps"] = sm_pool.tile([P, 512], fp32, tag="sm", name="ps")
            ps = state["ps"]
            for cc in range(lo, hi):
                nc.tensor.matmul(
                    ps[:],
                    wqk[:, cc, wcol:wcol + 128],
                    xt[:, cc, ts(n5, 512)],
                    start=(cc == 0),
                    stop=(cc == CC - 1),
                )
            if hi == CC:
                if jc < 2:  # q: add bias
                    nc.vector.tensor_scalar_add(
                        qk[:, jc, ts(n5, 512)], ps[:], bq[:, jc:jc + 1]
                    )
                else:  # k: bias dropped (softmax-invariant)
                    nc.vector.tensor_copy(out=qk[:, jc, ts(n5, 512)], in_=ps[:])

        def qk_drip(jc, wcol, n5, doses=4):
            # one projection group as `doses` equal chunk doses
            state = {}
            step = CC // doses
            return [
                lambda lo=lo: qk_part(jc, wcol, n5, state, lo, lo + step)
                for lo in range(0, CC, step)
            ]

        def emit_startup_kq():
            # k(pair0, g0) and q(pair0, n5=0) in separate PSUM tiles,
            # chunk-interleaved so both stream as xt block-0 chunks land.
            psk = sm_pool.tile([P, 512], fp32, tag="sm")
            psq = pp_pool.tile([P, 512], fp32, tag="pp")
            for cc in range(CC):
                nc.tensor.matmul(
                    psk[:], wqk[:, cc, 256:384], xt[:, cc, 0:512],
                    start=(cc == 0), stop=(cc == CC - 1),
                )
                nc.tensor.matmul(
                    psq[:], wqk[:, cc, 0:128], xt[:, cc, 0:512],
                    start=(cc == 0), stop=(cc == CC - 1),
                )
            nc.vector.tensor_copy(out=qk[:, 2, 0:512], in_=psk[:])
            nc.vector.tensor_scalar_add(qk[:, 0, 0:512], psq[:], bq[:, 0:1])

        def v_pair(nt, pool=None):
            # V projection for two token-chunks sharing one PSUM tile in
            # the (attention-phase idle) pp bank: one eviction per pair.
            # Pairs alternate between the pp and sm banks so the ring
            # turn (matmuls + eviction) keeps up with the slot rate.
            # (pool size is per-tag: reuse the host pool's tag.)
            pool = pool or pp_pool
            tag = "sm" if pool is sm_pool else "pp"
            psv = pool.tile([P, 512], fp32, tag=tag, name="psv")
            for half in range(2):
                for cc in range(CC):
                    nc.tensor.matmul(
                        psv[:, ts(half, 256)],
                        xt[:, cc, ts(nt + half, 128)],
                        wv[:, cc, :],
                        start=(cc == 0),
                        stop=(cc == CC - 1),
                    )
            nc.vector.tensor_copy(
                out=vsb[:, nt:nt + 2, :, 0:64],
                in_=psv[:].rearrange("p (t h d) -> p t h d", t=2, d=D),
            )

        def emit_scores_exp(pc, n5, mc):
            s = s_pool.tile([P, 1024], fp32, tag="s")
            nc.tensor.matmul(
                s[:, 0:512],
                qk[0:64, 2 + pc, ts(mc, 128)],
                qk[0:64, pc, ts(n5, 512)],
                start=True, stop=True, tile_position=(0, 0),
            )
            nc.tensor.matmul(
                s[:, 512:1024],
                qk[64:128, 2 + pc, ts(mc, 128)],
                qk[64:128, pc, ts(n5, 512)],
                start=True, stop=True, tile_position=(64, 0),
            )
            pab = p_pool.tile([P, 1024], bf16, tag="pab")
            nc.scalar.activation(pab[:], s[:], Exp, scale=0.125)
            return pab

        def emit_pv(pc, mc, pab, o2a, o2b):
            nc.tensor.matmul(
                o2a[:], vsb[:, mc, 2 * pc, 0:65], pab[:, 0:512],
                start=(mc == 0), stop=(mc == MC - 1),
            )
            nc.tensor.matmul(
                o2b[:], vsb[:, mc, 2 * pc + 1, 0:65], pab[:, 512:1024],
                start=(mc == 0), stop=(mc == MC - 1),
            )

        def emit_norm(pc, n5, o2a, o2b, last=False):
            # evict + normalize: o2n[:, pc, n5] = (o2 / den) + bias_v.
            # Denominators (row 64) are partition-broadcast via a DRAM
            # bounce. The final block avoids the gpsimd queue: Tile parks
            # a multi-us DRAIN there at the tail.
            alt = nc.scalar if last else nc.gpsimd
            for hl, o2 in ((0, o2a), (1, o2b)):
                o2s = ev_pool.tile([65, 512], fp32, tag="o2s")
                nc.vector.tensor_copy(out=o2s[:], in_=o2[:])
                rd = dram_pool.tile([1, 512], fp32, tag="rd")
                eng = (alt, nc.sync)[hl]
                eng.dma_start(rd[:], o2s[64:65, :])
                rb = ev_pool.tile([64, 512], fp32, tag="rb")
                rd_bcast = bass.AP(
                    tensor=rd.tensor, offset=rd.offset, ap=[[0, 64], [1, 512]]
                )
                eng.dma_start(rb[:], rd_bcast)
                nc.vector.reciprocal_approx_fast(out=rb[:], in_=rb[:])
                if hl == 0:
                    dst = o2n[0:64, pc, ts(n5, 512)]
                    nc.vector.scalar_tensor_tensor(
                        dst, o2s[0:64, :], 1.0, rb[:], op0=mult, op1=mult
                    )
                    nc.vector.tensor_scalar_add(
                        dst, dst, bv[:, 2 * pc:2 * pc + 1]
                    )
                else:
                    stg = ev_pool.tile([64, 512], bf16, tag="stg")
                    nc.vector.scalar_tensor_tensor(
                        stg[:], o2s[0:64, :], 1.0, rb[:], op0=mult, op1=mult
                    )
                    nc.vector.tensor_scalar_add(
                        stg[:], stg[:], bv[:, 2 * pc + 1:2 * pc + 2]
                    )
                    eng.dma_start(o2n[64:128, pc, ts(n5, 512)], stg[:])

        def emit_attn(pc, bg=None, bglow=None):
            # Software-pipelined: scores/exp for slot k+1 are emitted
            # before P@V of slot k so the exp stream stays dense; `bg`
            # drips run at natural priority, `bglow` at deep
            # deprioritization (scheduler picks them only in true slack).
            bg = bg or {}
            bglow = bglow or {}
            slots = [(n5, mc) for n5 in range(NC5) for mc in range(MC)]
            o2a = o2b = None
            pab_next = emit_scores_exp(pc, 0, 0)
            for fn in bg.get("pre", ()):
                fn()
            for k, (n5, mc) in enumerate(slots):
                pab = pab_next
                if mc == 0:
                    if n5 > 0:
                        emit_norm(pc, n5 - 1, o2a, o2b)
                    o2a = o2_pool.tile([65, 512], fp32, tag="o2")
                    o2b = o2_pool.tile([65, 512], fp32, tag="o2")
                if k + 1 < len(slots):
                    pab_next = emit_scores_exp(pc, *slots[k + 1])
                emit_pv(pc, mc, pab, o2a, o2b)
                for fn in bg.get((n5, mc), ()):
                    fn()
                with tc.high_priority(offset=-10_000_000):
                    for fn in bglow.get((n5, mc), ()):
                        fn()
            emit_norm(pc, NC5 - 1, o2a, o2b, last=(pc == 1))

        def proj_part(nt, state, ec):
            # one half of proj(nt) through the pp bank (idle once V is
            # done); dripped into attn(1) two passes after its o2n slice
            # settled, so the static schedule can't trap on it
            if "po" not in state:
                state["po"] = late.tile([P, 1024], fp32, tag="po", name="po")
            po = state["po"]
            pp = pp_pool.tile([P, 512], fp32, tag="pp", name="pp")
            for pc in range(2):
                nc.tensor.matmul(
                    pp[:], o2n[:, pc, ts(nt, 128)], wp[:, pc, ts(ec, 512)],
                    start=(pc == 0), stop=(pc == 1),
                )
            nc.vector.tensor_copy(out=po[:, ts(ec, 512)], in_=pp[:])
            if ec == 1:
                nc.sync.dma_start(out_d[ts(nt, 128), :], po[:])

        def proj_drip(nt):
            state = {}
            return [
                lambda: proj_part(nt, state, 0),
                lambda: proj_part(nt, state, 1),
            ]

        def emit_proj_nt_tail(nt, use_scalar):
            # tail drain: the scores pool is dead after the last exp, so
            # use its [128,1024] tiles (2 banks) and one wide eviction,
            # alternating between the two free copy engines
            pp2 = s_pool.tile([P, 1024], fp32, tag="s")
            for ec in range(2):
                for pc in range(2):
                    nc.tensor.matmul(
                        pp2[:, ts(ec, 512)],
                        o2n[:, pc, ts(nt, 128)], wp[:, pc, ts(ec, 512)],
                        start=(pc == 0), stop=(pc == 1),
                    )
            po = late.tile([P, 1024], fp32, tag="po", name="po")
            if use_scalar:
                nc.scalar.activation(po[:], pp2[:], Copy)
            else:
                nc.vector.tensor_copy(out=po[:], in_=pp2[:])
            nc.sync.dma_start(out_d[ts(nt, 128), :], po[:])

        # drip schedules ------------------------------------------------
        def sched(bg, key, fns):
            n5, mc = key
            for i, fn in enumerate(fns):
                bg.setdefault((n5, mc + i), []).append(fn)

        def vfn(nt, pool=None):
            return lambda: v_pair(nt, pool)

        bg0 = {"pre": [vfn(0)]}
        for j in range(7):                         # V pairs (2,3)...(14,15)
            pool = sm_pool if (j % 2 == 0) else pp_pool
            sched(bg0, (0, 2 * j), [vfn(2 * j + 2, pool)])
        for g in (1, 2, 3):                        # k pair0 g=1..3
            sched(bg0, (0, 4 * g - 3), qk_drip(2, 256, g, doses=2))
        for n5 in range(NC5 - 1):                  # q pair0 n5+1
            sched(bg0, (n5, 10), qk_drip(0, 0, n5 + 1))
        sched(bg0, (NC5 - 1, 2), qk_drip(3, 384, 0))   # k pair1 g=0
        sched(bg0, (NC5 - 1, 6), qk_drip(1, 128, 0))   # q pair1 n5=0

        bg1 = {}
        for g in (1, 2, 3):                        # k pair1 g=1..3
            sched(bg1, (0, 4 * g - 4), qk_drip(3, 384, g))
        for n5 in range(NC5 - 1):                  # q pair1 n5+1
            sched(bg1, (n5, 10), qk_drip(1, 128, n5 + 1))
        bg1low = {}
        for i in range(8):                         # proj nt 0-7, 2 passes late
            sched(bg1low, (2 + i // 4, 2 * (i % 4) + 1), proj_drip(i))
        for i in range(3):                         # proj nt 8-10, 1 pass late
            sched(bg1low, (3, 9 + 2 * i), proj_drip(8 + i))

        # PE warm-up: ~4us of throwaway matmuls while the first DMAs are
        # in flight, so the HAM clock-gate reaches 2.4GHz before the
        # first projection groups run (cold MMs run at half rate).
        wrm = persist.tile([P, 128], bf16)
        nc.vector.memset(wrm[:], 0.5)
        wps = sm_pool.tile([P, 512], fp32, tag="sm")
        for _ in range(24):
            nc.tensor.matmul(wps[:, 0:128], wrm[:], wrm[:], start=True, stop=True)

        emit_startup_kq()
        emit_attn(0, bg0)
        emit_attn(1, bg1, bg1low)
        for i, nt in enumerate(range(11, MC)):
            emit_proj_nt_tail(nt, use_scalar=(i % 2 == 1))

    nc.compile()
    return nc


def _get_built(n_tok):
    if n_tok not in _BUILT:
        _BUILT[n_tok] = _build(n_tok)
    return _BUILT[n_tok]


def make_in_map(x_b, w_qkv, b_qkv, w_proj, g):
    """Per-core input shards: batch slice x_b, head-group g (4 heads)."""
    import ml_dtypes

    f = np.float32
    bf = ml_dtypes.bfloat16
    cq = slice(g * 256, g * 256 + 256)
    ck = slice(C + g * 256, C + g * 256 + 256)
    cv = slice(2 * C + g * 256, 2 * C + g * 256 + 256)
    return {
        "xt": np.ascontiguousarray(np.asarray(x_b, f).T.astype(bf)),
        "w_qk": np.ascontiguousarray(
            np.concatenate(
                [np.asarray(w_qkv[:, cq], f), np.asarray(w_qkv[:, ck], f)], axis=1
            ).astype(bf)
        ),
        "w_v": np.ascontiguousarray(np.asarray(w_qkv[:, cv], f).astype(bf)),
        "w_p": np.ascontiguousarray(
            np.asarray(w_proj[g * 256:(g + 1) * 256, :], f).astype(bf)
        ),
        "b_q": np.ascontiguousarray(np.asarray(b_qkv[cq], f)),
        "b_v2": np.ascontiguousarray(np.asarray(b_qkv[cv], f).reshape(4, 64).T),
    }


def kernel(x, w_qkv, b_qkv, w_proj, b_proj):
    from concourse.bass_utils import run_bass_kernel_spmd

    x = np.asarray(x, np.float32)
    B, n_tok, _ = x.shape
    nc = _get_built(n_tok)

    in_maps = [
        make_in_map(x[core // 4], w_qkv, b_qkv, w_proj, core % 4)
        for core in range(N_CORES)
    ]
    res = run_bass_kernel_spmd(
        nc, in_maps, core_ids=list(range(N_CORES)), trace=TRACE
    )
    global LAST_RESULTS
    LAST_RESULTS = res
    outs = [r["out"] for r in res.results]
    bp = np.asarray(b_proj, np.float32)
    full = np.stack(
        [
            outs[4 * b] + outs[4 * b + 1] + outs[4 * b + 2] + outs[4 * b + 3] + bp
            for b in range(B)
        ]
    )
    return full.astype(np.float32)


# revision 52
# speedup vs baseline: 1.2105x; 1.2105x over previous
"""Tensor-parallel multi-head attention for Trainium2 (8 NeuronCores).

Problem: nn_MultiHeadAttention (B=2, N=2048, C=1024, H=16, D=64), fp32 in/out.

Sharding: core = batch * 4 + head_group; each core handles 1 batch and 4
heads (tensor-parallel over heads, data-parallel over batch). Each core
computes its heads' QKV projections, attention, and a *partial* output
projection (its 256 rows of w_proj); the host sums the 4 partials per
batch and adds b_proj.

Kernel notes (bf16 matmuls, fp32 PSUM accumulation + softmax chain):
  - The Scalar engine's exp stream is the hard floor (1 elem/cycle/lane,
    dtype-independent): 128 ACTIVATEs of [128,1024] ~= 147us. Everything
    else is scheduled around keeping that stream dense.
  - x is transposed on the host to xT [C, N] (bf16); xT is DMA'd in
    token-blocks (block 0 chunk-by-chunk) so the first k/q projection
    groups stream in as data lands and the first exp fires early.
  - Scores are computed transposed: sT[m, n] = k[m].q[n], keys m on
    partitions -- P@V then needs no transposes. Two heads run
    concurrently on the PE via row-tiling (K=64 each).
  - Softmax: no max-subtraction (logits ~ N(0,1)); the denominator is
    obtained by appending a ones-column to V (row 64 of the [65,512]
    attention-output accumulators); probabilities are normalized after
    the P@V matmuls via a DRAM-bounce partition-broadcast + fast
    reciprocal; v-bias is added then (softmax rows sum to 1); k-bias is
    softmax-invariant and dropped; q-bias applied at QKV eviction;
    proj-bias added on the host.
  - Background work (V projection, k/q groups for later passes) drips
    into the attention slots in small doses; V uses a private PSUM bank
    (shared with the projection later -- disjoint phases) so its ring
    never couples with the k/q group ring.
  - The output projection mostly runs in the tail, where the scores
    pool is dead: wide [128,1024] PSUM tiles, evictions alternating
    between the two then-idle copy engines.
"""

import numpy as np
from contextlib import ExitStack

P = 128
C = 1024
D = 64
N_CORES = 8

_BUILT = {}
TRACE = False   # set True (e.g. from test.py) to capture an NTFF profile
LAST_RESULTS = None  # BassKernelResults of the most recent kernel() call


def _build(n_tok, debug=False):
    import concourse.bass as bass
    import concourse.mybir as mybir
    import concourse.tile as tile
    from concourse import bacc
    from concourse.bass import ts

    fp32 = mybir.dt.float32
    bf16 = mybir.dt.bfloat16
    Exp = mybir.ActivationFunctionType.Exp
    Copy = mybir.ActivationFunctionType.Copy
    mult = mybir.AluOpType.mult

    NC5 = n_tok // 512  # 512-wide query chunks
    MC = n_tok // 128   # 128-wide key chunks
    CC = C // P         # contraction chunks for projections

    nc = bacc.Bacc("TRN2", target_bir_lowering=False, debug=debug)

    xt_d = nc.dram_tensor("xt", [C, n_tok], bf16, kind="ExternalInput").ap()
    wqk_d = nc.dram_tensor("w_qk", [C, 512], bf16, kind="ExternalInput").ap()
    wv_d = nc.dram_tensor("w_v", [C, 256], bf16, kind="ExternalInput").ap()
    wp_d = nc.dram_tensor("w_p", [256, C], bf16, kind="ExternalInput").ap()
    bq_d = nc.dram_tensor("b_q", [256], fp32, kind="ExternalInput").ap()
    bv_d = nc.dram_tensor("b_v2", [64, 4], fp32, kind="ExternalInput").ap()
    out_d = nc.dram_tensor("out", [n_tok, C], fp32, kind="ExternalOutput").ap()

    with tile.TileContext(nc) as tc, ExitStack() as ctx:
        persist = ctx.enter_context(tc.tile_pool(name="persist", bufs=1))
        p_pool = ctx.enter_context(tc.tile_pool(name="p_pool", bufs=4))
        ev_pool = ctx.enter_context(tc.tile_pool(name="ev_pool", bufs=2))
        late = ctx.enter_context(tc.tile_pool(name="late", bufs=3))
        s_pool = ctx.enter_context(tc.tile_pool(name="s", bufs=2, space="PSUM"))
        o2_pool = ctx.enter_context(tc.tile_pool(name="o2", bufs=2, space="PSUM"))
        sm_pool = ctx.enter_context(tc.tile_pool(name="sm", bufs=1, space="PSUM"))
        pp_pool = ctx.enter_context(tc.tile_pool(name="pp", bufs=1, space="PSUM"))
        dram_pool = ctx.enter_context(tc.tile_pool(name="dram", bufs=4, space="DRAM"))

        xt = persist.tile([P, CC, n_tok], bf16)
        wqk = persist.tile([P, CC, 512], bf16)
        wv = persist.tile([P, CC, 256], bf16)
        bq = persist.tile([P, 2], fp32)
        wp = persist.tile([P, 2, C], bf16)
        bv = persist.tile([64, 4], fp32)
        qk = persist.tile([P, 4, n_tok], bf16)   # jc: 0,1 = qT pairs, 2,3 = kT pairs
        vsb = persist.tile([P, MC, 4, 65], bf16)  # token-major V + ones column
        o2n = persist.tile([P, 2, n_tok], bf16)  # normalized attn out, feature-major

        # DMA dispatch costs ~600ns/instruction, so keep the count low:
        # pair-0 weight columns first, then xt token-block 0 chunk-by-chunk
        # (per-chunk sems let the first k/q groups drip as data arrives),
        # then the rest as whole-block transfers; pair-1 weights and w_p
        # (not needed until attn(1)/proj) go last.
        # All transfers use whole DRAM rows per contraction chunk (one
        # contiguous multi-KB run per partition -> ~600ns dispatch each).
        # xt block 0 goes first in per-chunk pieces so the first k/q
        # groups stream in as chunks land; the row-remainders follow.
        xt_src = xt_d.rearrange("(co p) n -> p co n", p=P)
        wqk_src = wqk_d.rearrange("(co p) j -> p co j", p=P)
        nc.gpsimd.dma_start(bq[:], bq_d.rearrange("(c p) -> p c", p=P))
        nc.gpsimd.dma_start(bv[:], bv_d)
        for cc in range(CC):   # wqk whole rows (all of q/k, both pairs)
            eng = nc.sync if (cc % 2 == 0) else nc.scalar
            eng.dma_start(wqk[:, cc, :], wqk_src[:, cc, :])
        for cc in range(CC):   # xt block 0, per-chunk
            eng = nc.sync if (cc % 2 == 0) else nc.scalar
            eng.dma_start(xt[:, cc, 0:512], xt_src[:, cc, 0:512])
        nc.gpsimd.dma_start(wv[:], wv_d.rearrange("(co p) j -> p co j", p=P))
        for cc in range(CC):   # xt row remainders (tokens 512:end)
            eng = nc.sync if (cc % 2 == 0) else nc.scalar
            eng.dma_start(xt[:, cc, 512:n_tok], xt_src[:, cc, 512:n_tok])
        nc.gpsimd.dma_start(wp[:], wp_d.rearrange("(pc p) e -> p pc e", p=P))
        ones = persist.tile([P, 1], fp32)
        nc.vector.memset(ones[:], 1.0)
        nc.vector.tensor_copy(
            out=vsb[:, :, :, 64:65],
            in_=ones[:, None, :, None].to_broadcast((P, MC, 4, 1)),
        )

        def qk_part(jc, wcol, n5, state, lo, hi):
            # chunks [lo, hi) of one 512-wide projection group; evict at hi==CC
            if "ps" not in state:
                state["ps"] = sm_pool.tile([P, 512], fp32, tag="sm", name="ps")
            ps = state["ps"]
            for cc in range(lo, hi):
                nc.tensor.matmul(
                    ps[:],
                    wqk[:, cc, wcol:wcol + 128],
                    xt[:, cc, ts(n5, 512)],
                    start=(cc == 0),
                    stop=(cc == CC - 1),
                )
            if hi == CC:
                if jc < 2:  # q: add bias
                    nc.vector.tensor_scalar_add(
                        qk[:, jc, ts(n5, 512)], ps[:], bq[:, jc:jc + 1]
                    )
                else:  # k: bias dropped (softmax-invariant)
                    nc.vector.tensor_copy(out=qk[:, jc, ts(n5, 512)], in_=ps[:])

        def qk_drip(jc, wcol, n5, doses=4):
            # one projection group as `doses` equal chunk doses
            state = {}
            step = CC // doses
            return [
                lambda lo=lo: qk_part(jc, wcol, n5, state, lo, lo + step)
                for lo in range(0, CC, step)
            ]

        def emit_startup_kq():
            # k(pair0, g0) and q(pair0, n5=0) in separate PSUM tiles,
            # chunk-interleaved so both stream as xt block-0 chunks land.
            psk = sm_pool.tile([P, 512], fp32, tag="sm")
            psq = pp_pool.tile([P, 512], fp32, tag="pp")
            for cc in range(CC):
                nc.tensor.matmul(
                    psk[:], wqk[:, cc, 256:384], xt[:, cc, 0:512],
                    start=(cc == 0), stop=(cc == CC - 1),
                )
                nc.tensor.matmul(
                    psq[:], wqk[:, cc, 0:128], xt[:, cc, 0:512],
                    start=(cc == 0), stop=(cc == CC - 1),
                )
            nc.vector.tensor_copy(out=qk[:, 2, 0:512], in_=psk[:])
            nc.vector.tensor_scalar_add(qk[:, 0, 0:512], psq[:], bq[:, 0:1])

        def v_pair(nt, pool=None):
            # V projection for two token-chunks sharing one PSUM tile in
            # the (attention-phase idle) pp bank: one eviction per pair.
            # Pairs alternate between the pp and sm banks so the ring
            # turn (matmuls + eviction) keeps up with the slot rate.
            # (pool size is per-tag: reuse the host pool's tag.)
            pool = pool or pp_pool
            tag = "sm" if pool is sm_pool else "pp"
            psv = pool.tile([P, 512], fp32, tag=tag, name="psv")
            for half in range(2):
                for cc in range(CC):
                    nc.tensor.matmul(
                        psv[:, ts(half, 256)],
                        xt[:, cc, ts(nt + half, 128)],
                        wv[:, cc, :],
                        start=(cc == 0),
                        stop=(cc == CC - 1),
                    )
            nc.vector.tensor_copy(
                out=vsb[:, nt:nt + 2, :, 0:64],
                in_=psv[:].rearrange("p (t h d) -> p t h d", t=2, d=D),
            )

        def emit_scores_exp(pc, n5, mc):
            s = s_pool.tile([P, 1024], fp32, tag="s")
            nc.tensor.matmul(
                s[:, 0:512],
                qk[0:64, 2 + pc, ts(mc, 128)],
                qk[0:64, pc, ts(n5, 512)],
                start=True, stop=True, tile_position=(0, 0),
            )
            nc.tensor.matmul(
                s[:, 512:1024],
                qk[64:128, 2 + pc, ts(mc, 128)],
                qk[64:128, pc, ts(n5, 512)],
                start=True, stop=True, tile_position=(64, 0),
            )
            pab = p_pool.tile([P, 1024], bf16, tag="pab")
            nc.scalar.activation(pab[:], s[:], Exp, scale=0.125)
            return pab

        def emit_pv(pc, mc, pab, o2a, o2b):
            nc.tensor.matmul(
                o2a[:], vsb[:, mc, 2 * pc, 0:65], pab[:, 0:512],
                start=(mc == 0), stop=(mc == MC - 1),
            )
            nc.tensor.matmul(
                o2b[:], vsb[:, mc, 2 * pc + 1, 0:65], pab[:, 512:1024],
                start=(mc == 0), stop=(mc == MC - 1),
            )

        def emit_norm(pc, n5, o2a, o2b, last=False):
            # evict + normalize: o2n[:, pc, n5] = (o2 / den) + bias_v.
            # Denominators (row 64) are partition-broadcast via a DRAM
            # bounce. The final block avoids the gpsimd queue: Tile parks
            # a multi-us DRAIN there at the tail.
            alt = nc.scalar if last else nc.gpsimd
            for hl, o2 in ((0, o2a), (1, o2b)):
                o2s = ev_pool.tile([65, 512], fp32, tag="o2s")
                nc.vector.tensor_copy(out=o2s[:], in_=o2[:])
                rd = dram_pool.tile([1, 512], fp32, tag="rd")
                eng = (alt, nc.sync)[hl]
                eng.dma_start(rd[:], o2s[64:65, :])
                rb = ev_pool.tile([64, 512], fp32, tag="rb")
                rd_bcast = bass.AP(
                    tensor=rd.tensor, offset=rd.offset, ap=[[0, 64], [1, 512]]
                )
                eng.dma_start(rb[:], rd_bcast)
                nc.vector.reciprocal_approx_fast(out=rb[:], in_=rb[:])
                if hl == 0:
                    dst = o2n[0:64, pc, ts(n5, 512)]
                    nc.vector.scalar_tensor_tensor(
                        dst, o2s[0:64, :], 1.0, rb[:], op0=mult, op1=mult
                    )
                    nc.vector.tensor_scalar_add(
                        dst, dst, bv[:, 2 * pc:2 * pc + 1]
                    )
                else:
                    stg = ev_pool.tile([64, 512], bf16, tag="stg")
                    nc.vector.scalar_tensor_tensor(
                        stg[:], o2s[0:64, :], 1.0, rb[:], op0=mult, op1=mult
                    )
                    nc.vector.tensor_scalar_add(
                        stg[:], stg[:], bv[:, 2 * pc + 1:2 * pc + 2]
                    )
                    eng.dma_start(o2n[64:128, pc, ts(n5, 512)], stg[:])

        def emit_attn(pc, bg=None, bglow=None):
            # Software-pipelined: scores/exp for slot k+1 are emitted
            # before P@V of slot k so the exp stream stays dense; `bg`
            # drips run at natural priority, `bglow` at deep
            # deprioritization (scheduler picks them only in true slack).
            bg = bg or {}
            bglow = bglow or {}
            slots = [(n5, mc) for n5 in range(NC5) for mc in range(MC)]
            o2a = o2b = None
            pab_next = emit_scores_exp(pc, 0, 0)
            for fn in bg.get("pre", ()):
                fn()
            for k, (n5, mc) in enumerate(slots):
                pab = pab_next
                if mc == 0:
                    if n5 > 0:
                        emit_norm(pc, n5 - 1, o2a, o2b)
                    o2a = o2_pool.tile([65, 512], fp32, tag="o2")
                    o2b = o2_pool.tile([65, 512], fp32, tag="o2")
                if k + 1 < len(slots):
                    pab_next = emit_scores_exp(pc, *slots[k + 1])
                emit_pv(pc, mc, pab, o2a, o2b)
                for fn in bg.get((n5, mc), ()):
                    fn()
                with tc.high_priority(offset=-10_000_000):
                    for fn in bglow.get((n5, mc), ()):
                        fn()
            emit_norm(pc, NC5 - 1, o2a, o2b, last=(pc == 1))

        def proj_part(nt, state, ec):
            # one half of proj(nt) through the pp bank (idle once V is
            # done); dripped into attn(1) two passes after its o2n slice
            # settled, so the static schedule can't trap on it
            if "po" not in state:
                state["po"] = late.tile([P, 1024], fp32, tag="po", name="po")
            po = state["po"]
            pp = pp_pool.tile([P, 512], fp32, tag="pp", name="pp")
            for pc in range(2):
                nc.tensor.matmul(
                    pp[:], o2n[:, pc, ts(nt, 128)], wp[:, pc, ts(ec, 512)],
                    start=(pc == 0), stop=(pc == 1),
                )
            nc.vector.tensor_copy(out=po[:, ts(ec, 512)], in_=pp[:])
            if ec == 1:
                nc.sync.dma_start(out_d[ts(nt, 128), :], po[:])

        def proj_drip(nt):
            state = {}
            return [
                lambda: proj_part(nt, state, 0),
                lambda: proj_part(nt, state, 1),
            ]

        def emit_proj_nt_tail(nt, use_scalar):
            # tail drain: the scores pool is dead after the last exp, so
            # use its [128,1024] tiles (2 banks) and one wide eviction,
            # alternating between the two free copy engines
            pp2 = s_pool.tile([P, 1024], fp32, tag="s")
            for ec in range(2):
                for pc in range(2):
                    nc.tensor.matmul(
                        pp2[:, ts(ec, 512)],
                        o2n[:, pc, ts(nt, 128)], wp[:, pc, ts(ec, 512)],
                        start=(pc == 0), stop=(pc == 1),
                    )
            po = late.tile([P, 1024], fp32, tag="po", name="po")
            if use_scalar:
                nc.scalar.activation(po[:], pp2[:], Copy)
            else:
                nc.vector.tensor_copy(out=po[:], in_=pp2[:])
            nc.sync.dma_start(out_d[ts(nt, 128), :], po[:])

        # drip schedules ------------------------------------------------
        def sched(bg, key, fns):
            n5, mc = key
            for i, fn in enumerate(fns):
                bg.setdefault((n5, mc + i), []).append(fn)

        def vfn(nt, pool=None):
            return lambda: v_pair(nt, pool)

        bg0 = {"pre": [vfn(0)]}
        for j in range(7):                         # V pairs (2,3)...(14,15)
            sched(bg0, (0, 2 * j), [vfn(2 * j + 2)])
        for g in (1, 2, 3):                        # k pair0 g=1..3
            sched(bg0, (0, 4 * g - 3), qk_drip(2, 256, g, doses=2))
        for n5 in range(NC5 - 1):                  # q pair0 n5+1
            sched(bg0, (n5, 10), qk_drip(0, 0, n5 + 1))
        sched(bg0, (NC5 - 1, 2), qk_drip(3, 384, 0))   # k pair1 g=0
        sched(bg0, (NC5 - 1, 6), qk_drip(1, 128, 0))   # q pair1 n5=0

        bg1 = {}
        for g in (1, 2, 3):                        # k pair1 g=1..3
            sched(bg1, (0, 4 * g - 4), qk_drip(3, 384, g))
        for n5 in range(NC5 - 1):                  # q pair1 n5+1
            sched(bg1, (n5, 10), qk_drip(1, 128, n5 + 1))
        bg1low = {}
        for i in range(8):                         # proj nt 0-7, 2 passes late
            sched(bg1low, (2 + i // 4, 2 * (i % 4) + 1), proj_drip(i))
        for i in range(3):                         # proj nt 8-10, 1 pass late
            sched(bg1low, (3, 9 + 2 * i), proj_drip(8 + i))

        # PE warm-up: ~4us of throwaway matmuls while the first DMAs are
        # in flight, so the HAM clock-gate reaches 2.4GHz before the
        # first projection groups run (cold MMs run at half rate).
        wrm = persist.tile([P, 128], bf16)
        nc.vector.memset(wrm[:], 0.5)
        wps = sm_pool.tile([P, 512], fp32, tag="sm")
        for _ in range(24):
            nc.tensor.matmul(wps[:, 0:128], wrm[:], wrm[:], start=True, stop=True)

        emit_startup_kq()
        emit_attn(0, bg0)
        emit_attn(1, bg1, bg1low)
        for i, nt in enumerate(range(11, MC)):
            emit_proj_nt_tail(nt, use_scalar=(i % 2 == 1))

    nc.compile()
    return nc


def _get_built(n_tok):
    if n_tok not in _BUILT:
        _BUILT[n_tok] = _build(n_tok)
    return _BUILT[n_tok]


def make_in_map(x_b, w_qkv, b_qkv, w_proj, g):
    """Per-core input shards: batch slice x_b, head-group g (4 heads)."""
    import ml_dtypes

    f = np.float32
    bf = ml_dtypes.bfloat16
    cq = slice(g * 256, g * 256 + 256)
    ck = slice(C + g * 256, C + g * 256 + 256)
    cv = slice(2 * C + g * 256, 2 * C + g * 256 + 256)
    return {
        "xt": np.ascontiguousarray(np.asarray(x_b, f).T.astype(bf)),
        "w_qk": np.ascontiguousarray(
            np.concatenate(
                [np.asarray(w_qkv[:, cq], f), np.asarray(w_qkv[:, ck], f)], axis=1
            ).astype(bf)
        ),
        "w_v": np.ascontiguousarray(np.asarray(w_qkv[:, cv], f).astype(bf)),
        "w_p": np.ascontiguousarray(
            np.asarray(w_proj[g * 256:(g + 1) * 256, :], f).astype(bf)
        ),
        "b_q": np.ascontiguousarray(np.asarray(b_qkv[cq], f)),
        "b_v2": np.ascontiguousarray(np.asarray(b_qkv[cv], f).reshape(4, 64).T),
    }


def kernel(x, w_qkv, b_qkv, w_proj, b_proj):
    from concourse.bass_utils import run_bass_kernel_spmd

    x = np.asarray(x, np.float32)
    B, n_tok, _ = x.shape
    nc = _get_built(n_tok)

    in_maps = [
        make_in_map(x[core // 4], w_qkv, b_qkv, w_proj, core % 4)
        for core in range(N_CORES)
    ]
    res = run_bass_kernel_spmd(
        nc, in_maps, core_ids=list(range(N_CORES)), trace=TRACE
    )
    global LAST_RESULTS
    LAST_RESULTS = res
    outs = [r["out"] for r in res.results]
    bp = np.asarray(b_proj, np.float32)
    full = np.stack(
        [
            outs[4 * b] + outs[4 * b + 1] + outs[4 * b + 2] + outs[4 * b + 3] + bp
            for b in range(B)
        ]
    )
    return full.astype(np.float32)


# revision 54
# speedup vs baseline: 1.2247x; 1.0117x over previous
"""Tensor-parallel multi-head attention for Trainium2 (8 NeuronCores).

Problem: nn_MultiHeadAttention (B=2, N=2048, C=1024, H=16, D=64), fp32 in/out.

Sharding: core = batch * 4 + head_group; each core handles 1 batch and 4
heads (tensor-parallel over heads, data-parallel over batch). Each core
computes its heads' QKV projections, attention, and a *partial* output
projection (its 256 rows of w_proj); the host sums the 4 partials per
batch and adds b_proj.

Kernel notes (bf16 matmuls, fp32 PSUM accumulation + softmax chain):
  - The Scalar engine's exp stream is the hard floor (1 elem/cycle/lane,
    dtype-independent): 128 ACTIVATEs of [128,1024] ~= 147us. Everything
    else is scheduled around keeping that stream dense.
  - x is transposed on the host to xT [C, N] (bf16); xT is DMA'd in
    token-blocks (block 0 chunk-by-chunk) so the first k/q projection
    groups stream in as data lands and the first exp fires early.
  - Scores are computed transposed: sT[m, n] = k[m].q[n], keys m on
    partitions -- P@V then needs no transposes. Two heads run
    concurrently on the PE via row-tiling (K=64 each).
  - Softmax: no max-subtraction (logits ~ N(0,1)); the denominator is
    obtained by appending a ones-column to V (row 64 of the [65,512]
    attention-output accumulators); probabilities are normalized after
    the P@V matmuls via a DRAM-bounce partition-broadcast + fast
    reciprocal; v-bias is added then (softmax rows sum to 1); k-bias is
    softmax-invariant and dropped; q-bias applied at QKV eviction;
    proj-bias added on the host.
  - Background work (V projection, k/q groups for later passes) drips
    into the attention slots in small doses; V uses a private PSUM bank
    (shared with the projection later -- disjoint phases) so its ring
    never couples with the k/q group ring.
  - The output projection mostly runs in the tail, where the scores
    pool is dead: wide [128,1024] PSUM tiles, evictions alternating
    between the two then-idle copy engines.
"""

import numpy as np
from contextlib import ExitStack

P = 128
C = 1024
D = 64
N_CORES = 8

_BUILT = {}
TRACE = False   # set True (e.g. from test.py) to capture an NTFF profile
LAST_RESULTS = None  # BassKernelResults of the most recent kernel() call


def _build(n_tok, debug=False):
    import concourse.bass as bass
    import concourse.mybir as mybir
    import concourse.tile as tile
    from concourse import bacc
    from concourse.bass import ts

    fp32 = mybir.dt.float32
    bf16 = mybir.dt.bfloat16
    Exp = mybir.ActivationFunctionType.Exp
    Copy = mybir.ActivationFunctionType.Copy
    mult = mybir.AluOpType.mult

    NC5 = n_tok // 512  # 512-wide query chunks
    MC = n_tok // 128   # 128-wide key chunks
    CC = C // P         # contraction chunks for projections

    nc = bacc.Bacc("TRN2", target_bir_lowering=False, debug=debug)

    xt_d = nc.dram_tensor("xt", [C, n_tok], bf16, kind="ExternalInput").ap()
    wqk_d = nc.dram_tensor("w_qk", [C, 512], bf16, kind="ExternalInput").ap()
    wv_d = nc.dram_tensor("w_v", [C, 256], bf16, kind="ExternalInput").ap()
    wp_d = nc.dram_tensor("w_p", [256, C], bf16, kind="ExternalInput").ap()
    bq_d = nc.dram_tensor("b_q", [256], fp32, kind="ExternalInput").ap()
    bv_d = nc.dram_tensor("b_v2", [64, 4], fp32, kind="ExternalInput").ap()
    out_d = nc.dram_tensor("out", [n_tok, C], fp32, kind="ExternalOutput").ap()

    with tile.TileContext(nc) as tc, ExitStack() as ctx:
        persist = ctx.enter_context(tc.tile_pool(name="persist", bufs=1))
        p_pool = ctx.enter_context(tc.tile_pool(name="p_pool", bufs=4))
        ev_pool = ctx.enter_context(tc.tile_pool(name="ev_pool", bufs=2))
        late = ctx.enter_context(tc.tile_pool(name="late", bufs=3))
        s_pool = ctx.enter_context(tc.tile_pool(name="s", bufs=2, space="PSUM"))
        o2_pool = ctx.enter_context(tc.tile_pool(name="o2", bufs=2, space="PSUM"))
        sm_pool = ctx.enter_context(tc.tile_pool(name="sm", bufs=1, space="PSUM"))
        pp_pool = ctx.enter_context(tc.tile_pool(name="pp", bufs=1, space="PSUM"))
        dram_pool = ctx.enter_context(tc.tile_pool(name="dram", bufs=4, space="DRAM"))

        xt = persist.tile([P, CC, n_tok], bf16)
        wqk = persist.tile([P, CC, 512], bf16)
        wv = persist.tile([P, CC, 256], bf16)
        bq = persist.tile([P, 2], fp32)
        wp = persist.tile([P, 2, C], bf16)
        bv = persist.tile([64, 4], fp32)
        qk = persist.tile([P, 4, n_tok], bf16)   # jc: 0,1 = qT pairs, 2,3 = kT pairs
        vsb = persist.tile([P, MC, 4, 65], bf16)  # token-major V + ones column
        o2n = persist.tile([P, 2, n_tok], bf16)  # normalized attn out, feature-major

        # DMA dispatch costs ~600ns/instruction, so keep the count low:
        # pair-0 weight columns first, then xt token-block 0 chunk-by-chunk
        # (per-chunk sems let the first k/q groups drip as data arrives),
        # then the rest as whole-block transfers; pair-1 weights and w_p
        # (not needed until attn(1)/proj) go last.
        # All transfers use whole DRAM rows per contraction chunk (one
        # contiguous multi-KB run per partition -> ~600ns dispatch each).
        # xt block 0 goes first in per-chunk pieces so the first k/q
        # groups stream in as chunks land; the row-remainders follow.
        xt_src = xt_d.rearrange("(co p) n -> p co n", p=P)
        wqk_src = wqk_d.rearrange("(co p) j -> p co j", p=P)
        nc.gpsimd.dma_start(bq[:], bq_d.rearrange("(c p) -> p c", p=P))
        nc.gpsimd.dma_start(bv[:], bv_d)
        for cc in range(CC):   # wqk whole rows (all of q/k, both pairs)
            eng = nc.sync if (cc % 2 == 0) else nc.scalar
            eng.dma_start(wqk[:, cc, :], wqk_src[:, cc, :])
        for cc in range(CC):   # xt block 0, per-chunk
            eng = nc.sync if (cc % 2 == 0) else nc.scalar
            eng.dma_start(xt[:, cc, 0:512], xt_src[:, cc, 0:512])
        nc.gpsimd.dma_start(wv[:], wv_d.rearrange("(co p) j -> p co j", p=P))
        for cc in range(CC):   # xt row remainders (tokens 512:end)
            eng = nc.sync if (cc % 2 == 0) else nc.scalar
            eng.dma_start(xt[:, cc, 512:n_tok], xt_src[:, cc, 512:n_tok])
        nc.gpsimd.dma_start(wp[:], wp_d.rearrange("(pc p) e -> p pc e", p=P))
        ones = persist.tile([P, 1], fp32)
        nc.vector.memset(ones[:], 1.0)
        nc.vector.tensor_copy(
            out=vsb[:, :, :, 64:65],
            in_=ones[:, None, :, None].to_broadcast((P, MC, 4, 1)),
        )

        def qk_part(jc, wcol, n5, state, lo, hi):
            # chunks [lo, hi) of one 512-wide projection group; evict at hi==CC
            if "ps" not in state:
                state["ps"] = sm_pool.tile([P, 512], fp32, tag="sm", name="ps")
            ps = state["ps"]
            for cc in range(lo, hi):
                nc.tensor.matmul(
                    ps[:],
                    wqk[:, cc, wcol:wcol + 128],
                    xt[:, cc, ts(n5, 512)],
                    start=(cc == 0),
                    stop=(cc == CC - 1),
                )
            if hi == CC:
                if jc < 2:  # q: add bias
                    nc.vector.tensor_scalar_add(
                        qk[:, jc, ts(n5, 512)], ps[:], bq[:, jc:jc + 1]
                    )
                else:  # k: bias dropped (softmax-invariant)
                    nc.vector.tensor_copy(out=qk[:, jc, ts(n5, 512)], in_=ps[:])

        def qk_drip(jc, wcol, n5, doses=4):
            # one projection group as `doses` equal chunk doses
            state = {}
            step = CC // doses
            return [
                lambda lo=lo: qk_part(jc, wcol, n5, state, lo, lo + step)
                for lo in range(0, CC, step)
            ]

        def emit_startup_kq():
            # k(pair0, g0) and q(pair0, n5=0) in separate PSUM tiles,
            # chunk-interleaved so both stream as xt block-0 chunks land.
            psk = sm_pool.tile([P, 512], fp32, tag="sm")
            psq = pp_pool.tile([P, 512], fp32, tag="pp")
            for cc in range(CC):
                nc.tensor.matmul(
                    psk[:], wqk[:, cc, 256:384], xt[:, cc, 0:512],
                    start=(cc == 0), stop=(cc == CC - 1),
                )
                nc.tensor.matmul(
                    psq[:], wqk[:, cc, 0:128], xt[:, cc, 0:512],
                    start=(cc == 0), stop=(cc == CC - 1),
                )
            nc.vector.tensor_copy(out=qk[:, 2, 0:512], in_=psk[:])
            nc.vector.tensor_scalar_add(qk[:, 0, 0:512], psq[:], bq[:, 0:1])

        def v_pair(nt, pool=None):
            # V projection for two token-chunks sharing one PSUM tile in
            # the (attention-phase idle) pp bank: one eviction per pair.
            # Pairs alternate between the pp and sm banks so the ring
            # turn (matmuls + eviction) keeps up with the slot rate.
            # (pool size is per-tag: reuse the host pool's tag.)
            pool = pool or pp_pool
            tag = "sm" if pool is sm_pool else "pp"
            psv = pool.tile([P, 512], fp32, tag=tag, name="psv")
            for half in range(2):
                for cc in range(CC):
                    nc.tensor.matmul(
                        psv[:, ts(half, 256)],
                        xt[:, cc, ts(nt + half, 128)],
                        wv[:, cc, :],
                        start=(cc == 0),
                        stop=(cc == CC - 1),
                    )
            nc.vector.tensor_copy(
                out=vsb[:, nt:nt + 2, :, 0:64],
                in_=psv[:].rearrange("p (t h d) -> p t h d", t=2, d=D),
            )

        def emit_scores_exp(pc, n5, mc):
            s = s_pool.tile([P, 1024], fp32, tag="s")
            nc.tensor.matmul(
                s[:, 0:512],
                qk[0:64, 2 + pc, ts(mc, 128)],
                qk[0:64, pc, ts(n5, 512)],
                start=True, stop=True, tile_position=(0, 0),
            )
            nc.tensor.matmul(
                s[:, 512:1024],
                qk[64:128, 2 + pc, ts(mc, 128)],
                qk[64:128, pc, ts(n5, 512)],
                start=True, stop=True, tile_position=(64, 0),
            )
            pab = p_pool.tile([P, 1024], bf16, tag="pab")
            nc.scalar.activation(pab[:], s[:], Exp, scale=0.125)
            return pab

        def emit_pv(pc, mc, pab, o2a, o2b):
            nc.tensor.matmul(
                o2a[:], vsb[:, mc, 2 * pc, 0:65], pab[:, 0:512],
                start=(mc == 0), stop=(mc == MC - 1),
            )
            nc.tensor.matmul(
                o2b[:], vsb[:, mc, 2 * pc + 1, 0:65], pab[:, 512:1024],
                start=(mc == 0), stop=(mc == MC - 1),
            )

        def emit_norm(pc, n5, o2a, o2b, last=False):
            # evict + normalize: o2n[:, pc, n5] = (o2 / den) + bias_v.
            # Denominators (row 64) are partition-broadcast via a DRAM
            # bounce. The final block avoids the gpsimd queue: Tile parks
            # a multi-us DRAIN there at the tail.
            alt = nc.scalar if last else nc.gpsimd
            for hl, o2 in ((0, o2a), (1, o2b)):
                o2s = ev_pool.tile([65, 512], fp32, tag="o2s")
                nc.vector.tensor_copy(out=o2s[:], in_=o2[:])
                rd = dram_pool.tile([1, 512], fp32, tag="rd")
                eng = (alt, nc.sync)[hl]
                eng.dma_start(rd[:], o2s[64:65, :])
                rb = ev_pool.tile([64, 512], fp32, tag="rb")
                rd_bcast = bass.AP(
                    tensor=rd.tensor, offset=rd.offset, ap=[[0, 64], [1, 512]]
                )
                eng.dma_start(rb[:], rd_bcast)
                nc.vector.reciprocal_approx_fast(out=rb[:], in_=rb[:])
                if hl == 0:
                    dst = o2n[0:64, pc, ts(n5, 512)]
                    nc.vector.scalar_tensor_tensor(
                        dst, o2s[0:64, :], 1.0, rb[:], op0=mult, op1=mult
                    )
                    nc.vector.tensor_scalar_add(
                        dst, dst, bv[:, 2 * pc:2 * pc + 1]
                    )
                else:
                    stg = ev_pool.tile([64, 512], bf16, tag="stg")
                    nc.vector.scalar_tensor_tensor(
                        stg[:], o2s[0:64, :], 1.0, rb[:], op0=mult, op1=mult
                    )
                    nc.vector.tensor_scalar_add(
                        stg[:], stg[:], bv[:, 2 * pc + 1:2 * pc + 2]
                    )
                    eng.dma_start(o2n[64:128, pc, ts(n5, 512)], stg[:])

        def emit_attn(pc, bg=None, bglow=None):
            # Software-pipelined: scores/exp for slot k+1 are emitted
            # before P@V of slot k so the exp stream stays dense; `bg`
            # drips run at natural priority, `bglow` at deep
            # deprioritization (scheduler picks them only in true slack).
            bg = bg or {}
            bglow = bglow or {}
            slots = [(n5, mc) for n5 in range(NC5) for mc in range(MC)]
            o2a = o2b = None
            pab_next = emit_scores_exp(pc, 0, 0)
            for fn in bg.get("pre", ()):
                fn()
            for k, (n5, mc) in enumerate(slots):
                pab = pab_next
                if mc == 0:
                    if n5 > 0:
                        emit_norm(pc, n5 - 1, o2a, o2b)
                    o2a = o2_pool.tile([65, 512], fp32, tag="o2")
                    o2b = o2_pool.tile([65, 512], fp32, tag="o2")
                if k + 1 < len(slots):
                    pab_next = emit_scores_exp(pc, *slots[k + 1])
                emit_pv(pc, mc, pab, o2a, o2b)
                for fn in bg.get((n5, mc), ()):
                    fn()
                with tc.high_priority(offset=-10_000_000):
                    for fn in bglow.get((n5, mc), ()):
                        fn()
            emit_norm(pc, NC5 - 1, o2a, o2b, last=(pc == 1))

        def proj_part(nt, state, ec):
            # one half of proj(nt) through the pp bank (idle once V is
            # done); dripped into attn(1) two passes after its o2n slice
            # settled, so the static schedule can't trap on it
            if "po" not in state:
                state["po"] = late.tile([P, 1024], fp32, tag="po", name="po")
            po = state["po"]
            pp = pp_pool.tile([P, 512], fp32, tag="pp", name="pp")
            for pc in range(2):
                nc.tensor.matmul(
                    pp[:], o2n[:, pc, ts(nt, 128)], wp[:, pc, ts(ec, 512)],
                    start=(pc == 0), stop=(pc == 1),
                )
            nc.vector.tensor_copy(out=po[:, ts(ec, 512)], in_=pp[:])
            if ec == 1:
                nc.sync.dma_start(out_d[ts(nt, 128), :], po[:])

        def proj_drip(nt):
            state = {}
            return [
                lambda: proj_part(nt, state, 0),
                lambda: proj_part(nt, state, 1),
            ]

        def emit_proj_nt_tail(nt, use_scalar):
            # tail drain: the scores pool is dead after the last exp, so
            # use its [128,1024] tiles (2 banks) and one wide eviction,
            # alternating between the two free copy engines
            pp2 = s_pool.tile([P, 1024], fp32, tag="s")
            for ec in range(2):
                for pc in range(2):
                    nc.tensor.matmul(
                        pp2[:, ts(ec, 512)],
                        o2n[:, pc, ts(nt, 128)], wp[:, pc, ts(ec, 512)],
                        start=(pc == 0), stop=(pc == 1),
                    )
            po = late.tile([P, 1024], fp32, tag="po", name="po")
            if use_scalar:
                nc.scalar.activation(po[:], pp2[:], Copy)
            else:
                nc.vector.tensor_copy(out=po[:], in_=pp2[:])
            nc.sync.dma_start(out_d[ts(nt, 128), :], po[:])

        # drip schedules ------------------------------------------------
        def sched(bg, key, fns):
            n5, mc = key
            for i, fn in enumerate(fns):
                bg.setdefault((n5, mc + i), []).append(fn)

        def vfn(nt, pool=None):
            return lambda: v_pair(nt, pool)

        bg0 = {"pre": [vfn(0)]}
        for j in range(7):                         # V pairs (2,3)...(14,15)
            sched(bg0, (0, 2 * j), [vfn(2 * j + 2)])
        for g in (1, 2, 3):                        # k pair0 g=1..3
            sched(bg0, (0, 4 * g - 3), qk_drip(2, 256, g, doses=2))
        for n5 in range(NC5 - 1):                  # q pair0 n5+1
            sched(bg0, (n5, 10), qk_drip(0, 0, n5 + 1))
        sched(bg0, (NC5 - 1, 2), qk_drip(3, 384, 0))   # k pair1 g=0
        sched(bg0, (NC5 - 1, 6), qk_drip(1, 128, 0))   # q pair1 n5=0

        bg1 = {}
        for g in (1, 2, 3):                        # k pair1 g=1..3
            sched(bg1, (0, 4 * g - 4), qk_drip(3, 384, g))
        for n5 in range(NC5 - 1):                  # q pair1 n5+1
            sched(bg1, (n5, 10), qk_drip(1, 128, n5 + 1))
        bg1low = {}
        for i in range(8):                         # proj nt 0-7, 2 passes late
            sched(bg1low, (2 + i // 4, 2 * (i % 4) + 1), proj_drip(i))

        # PE warm-up: ~4us of throwaway matmuls while the first DMAs are
        # in flight, so the HAM clock-gate reaches 2.4GHz before the
        # first projection groups run (cold MMs run at half rate).
        wrm = persist.tile([P, 128], bf16)
        nc.vector.memset(wrm[:], 0.5)
        wps = sm_pool.tile([P, 512], fp32, tag="sm")
        for _ in range(24):
            nc.tensor.matmul(wps[:, 0:128], wrm[:], wrm[:], start=True, stop=True)

        emit_startup_kq()
        emit_attn(0, bg0)
        emit_attn(1, bg1, bg1low)
        for i, nt in enumerate(range(8, MC)):
            emit_proj_nt_tail(nt, use_scalar=(i % 2 == 1))

    nc.compile()
    return nc


def _get_built(n_tok):
    if n_tok not in _BUILT:
        _BUILT[n_tok] = _build(n_tok)
    return _BUILT[n_tok]


def make_in_map(x_b, w_qkv, b_qkv, w_proj, g):
    """Per-core input shards: batch slice x_b, head-group g (4 heads)."""
    import ml_dtypes

    f = np.float32
    bf = ml_dtypes.bfloat16
    cq = slice(g * 256, g * 256 + 256)
    ck = slice(C + g * 256, C + g * 256 + 256)
    cv = slice(2 * C + g * 256, 2 * C + g * 256 + 256)
    return {
        "xt": np.ascontiguousarray(np.asarray(x_b, f).T.astype(bf)),
        "w_qk": np.ascontiguousarray(
            np.concatenate(
                [np.asarray(w_qkv[:, cq], f), np.asarray(w_qkv[:, ck], f)], axis=1
            ).astype(bf)
        ),
        "w_v": np.ascontiguousarray(np.asarray(w_qkv[:, cv], f).astype(bf)),
        "w_p": np.ascontiguousarray(
            np.asarray(w_proj[g * 256:(g + 1) * 256, :], f).astype(bf)
        ),
        "b_q": np.ascontiguousarray(np.asarray(b_qkv[cq], f)),
        "b_v2": np.ascontiguousarray(np.asarray(b_qkv[cv], f).reshape(4, 64).T),
    }


def kernel(x, w_qkv, b_qkv, w_proj, b_proj):
    from concourse.bass_utils import run_bass_kernel_spmd

    x = np.asarray(x, np.float32)
    B, n_tok, _ = x.shape
    nc = _get_built(n_tok)

    in_maps = [
        make_in_map(x[core // 4], w_qkv, b_qkv, w_proj, core % 4)
        for core in range(N_CORES)
    ]
    res = run_bass_kernel_spmd(
        nc, in_maps, core_ids=list(range(N_CORES)), trace=TRACE
    )
    global LAST_RESULTS
    LAST_RESULTS = res
    outs = [r["out"] for r in res.results]
    bp = np.asarray(b_proj, np.float32)
    full = np.stack(
        [
            outs[4 * b] + outs[4 * b + 1] + outs[4 * b + 2] + outs[4 * b + 3] + bp
            for b in range(B)
        ]
    )
    return full.astype(np.float32)
